# revision 38
# baseline (speedup 1.0000x reference)
"""Fused single-head attention + residual + LayerNorm for Trainium2 (Bass/Tile).

Problem: B=4, S=4096, E=512 fp32.
  Q/K/V = x @ W^T + b ; S = QK^T/sqrt(E) ; mask keys ; softmax ; ctx = P@V ;
  out = LayerNorm(ctx + x) * gamma + beta

Sharding: 8 cores = 4 batches x 2 halves of the S=4096 query rows
(sequence-parallel attention). Masked keys carry exactly zero softmax
weight, so each core receives only the PACKED (unmasked) key rows of its
whole batch -- replicated to both cores of the batch pair by the host,
which is cheaper and far more predictable than an on-device K/V exchange
at this size (a 2-core AllGather costs a ~25us CC-engine init plus
rendezvous jitter).

Kernel strategy:
  - All heavy matmuls in fp8e4 with MatmulPerfMode.DoubleRow: both
    operands are laid out [128, 2, free] so each matmul contracts 256
    rows at 0.5 PE cycles/row (2x the bf16 rate). The attention output
    ("context") is ~1.5% of the magnitude of the residual x, so fp8
    rounding in the attention path is strongly damped in the final
    output (measured rel-err ~2.7e-3 end to end, vs the 2e-2 gate).
  - Keys are packed at 128 granularity: nkt2 = ceil(unmasked/128)
    rounded even (DoubleRow pairs), typically 18 tiles vs the 32 of the
    unpacked S=4096.
  - The host pre-transposes and pre-quantizes the matmul operands:
    x^T and W arrive as fp8 in the paired layout and DMA straight into
    SBUF (no on-device transposes or casts). The residual x arrives
    bf16 with bv pre-added (exact: ctx/rs + bv + x == true ctx + x),
    and bk is dropped entirely (a per-query score constant, softmax-
    invariant). Output is written bf16 and upcast by the host.
  - Scores are computed transposed S^T[k, q] (k on partitions) into one
    PSUM tile [128, 4qc, 512] per k-tile, so ONE ScalarE activation
    exp(s*scale + maskbias_k - 1) covers all 2048 q for that k-tile
    (the -1 shift keeps exp < 8 for fp8 headroom; softmax normalization
    cancels it). P is written directly in fp8 paired layout for ctx.
  - Row sums ride in the P@V matmul via a ones-column appended to V.
  - Single ctx accumulation over all k-tiles (no spill/combine pass).
  - LayerNorm: h built by DVE scalar_tensor_tensor from PSUM, stats via
    bn_stats, sqrt on ScalarE (all sqrts happen after all exps -> one
    activation-table switch), normalize on ScalarE Identity with
    per-partition scale/bias.
"""

import sys

import ml_dtypes
import numpy as np

sys.path.insert(0, "/opt/trn_rl_repo")

import concourse.bass as bass  # noqa: E402
import concourse.tile as tile  # noqa: E402
from concourse import bacc, mybir  # noqa: E402

E = 512
S = 4096  # keys per batch
SQ = 2048  # query rows per core
QC = SQ // 512  # 4   512-chunks along q
F32 = mybir.dt.float32
BF16 = mybir.dt.bfloat16
FP8 = mybir.dt.float8e4
SCALE = 1.0 / float(np.sqrt(E))
EPS = 1e-5
MASK_NEG = -10000.0
SHIFT = -1.0  # softmax-invariant score shift, keeps exp() in fp8 range
DR = mybir.MatmulPerfMode.DoubleRow


def build_nc(nkt2, apply_gb):
    # nkt2 = k-tiles of 128 packed (unmasked) keys per batch, even
    assert nkt2 % 2 == 0
    SK = nkt2 * 128  # packed keys
    JP = nkt2 // 2  # ctx pair-tiles of 256 keys
    chunks = [(a, min(a + 512, SK)) for a in range(0, SK, 512)]

    nc = bacc.Bacc("TRN2", target_bir_lowering=False, debug=False)
    xq = nc.dram_tensor("xq", [SQ, E], BF16, kind="ExternalInput")
    xqT8 = nc.dram_tensor("xqT8", [E, SQ], FP8, kind="ExternalInput")
    xkvT8 = nc.dram_tensor("xkvT8", [E, SK], FP8, kind="ExternalInput")
    mbias = nc.dram_tensor("maskbias", [SK], F32, kind="ExternalInput")
    Wq8 = nc.dram_tensor("Wq8", [E, E], FP8, kind="ExternalInput")
    Wk8 = nc.dram_tensor("Wk8", [E, E], FP8, kind="ExternalInput")
    Wv8 = nc.dram_tensor("Wv8", [E, E], FP8, kind="ExternalInput")
    bq = nc.dram_tensor("bq", [E], F32, kind="ExternalInput")
    gamma = nc.dram_tensor("gamma", [E], F32, kind="ExternalInput")
    beta = nc.dram_tensor("beta", [E], F32, kind="ExternalInput")
    out = nc.dram_tensor("out", [SQ, E], BF16, kind="ExternalOutput")

    AF = mybir.ActivationFunctionType
    OP = mybir.AluOpType

    def ap3(handle, offset, dims):
        a = handle[:]
        return bass.AP(tensor=a.tensor, offset=offset, ap=dims)

    with tile.TileContext(nc) as tc:
        with tc.tile_pool(name="persist", bufs=1) as persist:
            # ---------------- constants ----------------
            mbcols = persist.tile([128, nkt2], F32, tag="mb")
            nc.gpsimd.dma_start(
                out=mbcols, in_=ap3(mbias, 0, [[1, 128], [128, nkt2]])
            )
            bqcol = persist.tile([128, 4], F32, tag="bq")
            nc.gpsimd.dma_start(out=bqcol, in_=ap3(bq, 0, [[1, 128], [128, 4]]))
            if apply_gb:
                ga_bc = persist.tile([128, E], F32, tag="gabc")
                be_bc = persist.tile([128, E], F32, tag="bebc")
                nc.gpsimd.dma_start(out=ga_bc, in_=ap3(gamma, 0, [[0, 128], [1, E]]))
                nc.gpsimd.dma_start(out=be_bc, in_=ap3(beta, 0, [[0, 128], [1, E]]))
            eps_t = persist.tile([128, 1], F32, tag="eps")
            nc.vector.memset(eps_t, EPS)

            # persistent fp8 operand tiles (paired [.., 2, ..] layouts)
            # w8[name]: [128e, 2(m), 2(i), 512f]; logical e = 256m + 128i + p
            w8 = {
                n: persist.tile([128, 2, 2, E], FP8, name=f"w8{n}", tag=f"w8{n}")
                for n in ("q", "k", "v")
            }
            # xq8[m]: [128e, 2(i), SQ]
            xq8 = [
                persist.tile([128, 2, SQ], FP8, name=f"xq8_{m}", tag=f"xq8_{m}")
                for m in range(2)
            ]
            # xkv8[m]: [128e, 2(i), SK]
            xkv8 = [
                persist.tile([128, 2, SK], FP8, name=f"xkv8_{m}", tag=f"xkv8_{m}")
                for m in range(2)
            ]
            # qT8[m][qc]: [128f, 2(i), 512q]   (f = 256m + 128i + p)
            qT8 = [
                [
                    persist.tile(
                        [128, 2, 512], FP8, name=f"qT8_{m}_{c}", tag=f"qT8_{m}_{c}"
                    )
                    for c in range(QC)
                ]
                for m in range(2)
            ]
            # kT8[m]: [128f, 2(i), SK]
            kT8 = [
                persist.tile([128, 2, SK], FP8, name=f"kT8_{m}", tag=f"kT8_{m}")
                for m in range(2)
            ]
            # v8[jp]: [128k, 2(i), 513]  (k = 256jp + 128i + p; col 512 = ones)
            v8 = [
                persist.tile([128, 2, E + 1], FP8, name=f"v8_{j}", tag=f"v8_{j}")
                for j in range(JP)
            ]
            # P tiles: pt[jp]: [128k, 4(qc), 2(i), 512q]
            pt = [
                persist.tile([128, 4, 2, 512], FP8, name=f"pt{j}", tag=f"pt{j}")
                for j in range(JP)
            ]
            # residual x kept staged for the LN phase
            xqst = [
                persist.tile([128, 4, E], BF16, name=f"xqst{c}", tag=f"xqst{c}")
                for c in range(QC)
            ]

            with tc.tile_pool(name="pr", bufs=6, space="PSUM") as prp:
                # bulk loads: x arrives pre-transposed AND pre-quantized to
                # the paired fp8 layout (host-side), W likewise -- the
                # operands DMA straight into their SBUF tiles, no staging,
                # no casts. sync: xkvT8 | W*8 | xqT8 | xq(residual)
                def wload(name, wdram):
                    nc.sync.dma_start(
                        out=w8[name],
                        in_=ap3(
                            wdram,
                            0,
                            [[512, 128], [512 * 256, 2], [512 * 128, 2], [1, 512]],
                        ),
                    )

                wload("q", Wq8)
                for m in range(2):
                    nc.sync.dma_start(
                        out=xq8[m],
                        in_=ap3(
                            xqT8,
                            m * 256 * SQ,
                            [[SQ, 128], [128 * SQ, 2], [1, SQ]],
                        ),
                    )
                wload("k", Wk8)
                wload("v", Wv8)
                for m in range(2):
                    nc.sync.dma_start(
                        out=xkv8[m],
                        in_=ap3(
                            xkvT8,
                            m * 256 * SK,
                            [[SK, 128], [128 * SK, 2], [1, SK]],
                        ),
                    )
                for qc in range(QC):
                    nc.sync.dma_start(
                        out=xqst[qc][:, 0:4, :],
                        in_=ap3(xq, qc * 512 * E, [[512, 128], [65536, 4], [1, 512]]),
                    )

                def kv_proj(a, b):
                    # K^T [f, s]: psum tile per f-tile ft
                    for ft in range(4):
                        ps = prp.tile([128, 512], F32, tag="pr")
                        for m in range(2):
                            nc.tensor.matmul(
                                ps[:, 0 : b - a],
                                w8["k"][:, m, :, ft * 128 : (ft + 1) * 128],
                                xkv8[m][:, :, a:b],
                                start=(m == 0),
                                stop=(m == 1),
                                perf_mode=DR,
                            )
                        nc.vector.tensor_copy(
                            kT8[ft // 2][:, ft % 2, a:b], ps[:, 0 : b - a]
                        )
                    # V [s, f] per k-tile t
                    for t in range(a // 128, b // 128):
                        ps = prp.tile([128, 512], F32, tag="pr")
                        for m in range(2):
                            nc.tensor.matmul(
                                ps,
                                xkv8[m][:, :, t * 128 : (t + 1) * 128],
                                w8["v"][:, m, :, :],
                                start=(m == 0),
                                stop=(m == 1),
                                perf_mode=DR,
                            )
                        if t % 2 == 0:
                            nc.vector.tensor_copy(v8[t // 2][:, t % 2, 0:E], ps)
                        else:
                            nc.scalar.copy(out=v8[t // 2][:, t % 2, 0:E], in_=ps)
                        if t % 2 == 1:
                            nc.vector.memset(v8[t // 2][:, :, E : E + 1], 1.0)

                # ---- Q projections first (their ScalarE bias-copies run
                # during the xkvT stream, off the exp critical path) ----
                for qc in range(QC):
                    for ft in range(4):
                        ps = prp.tile([128, 512], F32, tag="pr")
                        for m in range(2):
                            nc.tensor.matmul(
                                ps,
                                w8["q"][:, m, :, ft * 128 : (ft + 1) * 128],
                                xq8[m][:, :, qc * 512 : (qc + 1) * 512],
                                start=(m == 0),
                                stop=(m == 1),
                                perf_mode=DR,
                            )
                        nc.scalar.activation(
                            out=qT8[ft // 2][qc][:, ft % 2, :],
                            in_=ps,
                            func=AF.Identity,
                            bias=bqcol[:, ft : ft + 1],
                        )

                # K/V projections per chunk
                for a, b in chunks:
                    kv_proj(a, b)

            # ---------------- scores + exp (all k-tiles) ----------------
            with tc.tile_pool(name="sc", bufs=2, space="PSUM") as scp:
                for kt in range(nkt2):
                    sc = scp.tile([128, 4, 512], F32, tag="sc")
                    for m in range(2):
                        for qc in range(QC):
                            nc.tensor.matmul(
                                sc[:, qc, :],
                                kT8[m][:, :, kt * 128 : (kt + 1) * 128],
                                qT8[m][qc],
                                start=(m == 0),
                                stop=(m == 1),
                                perf_mode=DR,
                            )
                    nc.scalar.activation(
                        out=pt[kt // 2][:, :, kt % 2, :],
                        in_=sc,
                        func=AF.Exp,
                        bias=mbcols[:, kt : kt + 1],
                        scale=SCALE,
                    )

            # ---------------- ctx + residual + layernorm ----------------
            with (
                tc.tile_pool(name="cx", bufs=4, space="PSUM") as cxp,
                tc.tile_pool(name="wk", bufs=6) as work,
            ):
                for qi in range(16):
                    qc, st = qi // 4, qi % 4
                    cs = cxp.tile([128, 2, 512], F32, tag="cs")
                    for jp in range(JP):
                        lhs = pt[jp][:, qc, :, st * 128 : (st + 1) * 128]
                        nc.tensor.matmul(
                            cs[:, 0, 0:256],
                            lhs,
                            v8[jp][:, :, 0:256],
                            start=(jp == 0),
                            stop=(jp == JP - 1),
                            perf_mode=DR,
                        )
                        nc.tensor.matmul(
                            cs[:, 1, 0:257],
                            lhs,
                            v8[jp][:, :, 256 : E + 1],
                            start=(jp == 0),
                            stop=(jp == JP - 1),
                            perf_mode=DR,
                        )
                    recip = work.tile([128, 1], F32, tag="recip")
                    nc.vector.reciprocal(recip, cs[:, 1, 256:257])
                    h = work.tile([128, E], F32, tag="h")
                    nc.vector.scalar_tensor_tensor(
                        out=h[:, 0:256],
                        in0=cs[:, 0, 0:256],
                        scalar=recip,
                        in1=xqst[qc][:, st, 0:256],
                        op0=OP.mult,
                        op1=OP.add,
                    )
                    nc.vector.scalar_tensor_tensor(
                        out=h[:, 256:512],
                        in0=cs[:, 1, 0:256],
                        scalar=recip,
                        in1=xqst[qc][:, st, 256:512],
                        op0=OP.mult,
                        op1=OP.add,
                    )
                    st6 = work.tile([128, 6], F32, tag="st6")
                    nc.vector.bn_stats(out=st6, in_=h)
                    mv = work.tile([128, 2], F32, tag="mv")
                    nc.vector.bn_aggr(out=mv, in_=st6)
                    std = work.tile([128, 1], F32, tag="std")
                    nc.scalar.activation(
                        out=std, in_=mv[:, 1:2], func=AF.Sqrt, bias=eps_t
                    )
                    rstd = work.tile([128, 1], F32, tag="rstd")
                    nc.vector.reciprocal(rstd, std)
                    nmu = work.tile([128, 1], F32, tag="nmu")
                    nc.vector.tensor_scalar(
                        out=nmu,
                        in0=mv[:, 0:1],
                        scalar1=rstd,
                        scalar2=-1.0,
                        op0=OP.mult,
                        op1=OP.mult,
                    )
                    o_t = work.tile([128, E], BF16, tag="ot")
                    nc.scalar.activation(
                        out=o_t, in_=h, func=AF.Identity, bias=nmu, scale=rstd
                    )
                    if apply_gb:
                        nc.vector.tensor_mul(o_t, o_t, ga_bc)
                        nc.vector.tensor_add(o_t, o_t, be_bc)
                    nc.gpsimd.dma_start(
                        out=out[qi * 128 : (qi + 1) * 128, :], in_=o_t
                    )
    return nc


# test-harness knobs (the grading harness leaves these at defaults)
TRACE = False
LAST_RESULTS = None


def _ensure_axon_jax():
    """The Bass SPMD run goes through jax/PJRT on the axon platform. If the
    caller pinned jax to cpu (e.g. to run a reference model), unpin it and
    drop any initialized cpu-only backends."""
    import os

    import jax

    try:
        devs = jax.devices()
    except Exception:
        devs = []
    if any(d.platform not in ("cpu",) for d in devs):
        return
    os.environ.pop("JAX_PLATFORMS", None)
    try:
        jax.config.update("jax_platforms", None)
    except Exception:
        pass
    try:
        jax.clear_backends()
    except Exception:
        try:
            jax.extend.backend.clear_backends()
        except Exception:
            pass


def kernel(x, mask, Wq, bq, Wk, bk, Wv, bv, gamma, beta):
    global LAST_RESULTS
    _ensure_axon_jax()
    from concourse.bass_utils import run_bass_kernel_spmd

    x = np.ascontiguousarray(np.asarray(x, dtype=np.float32))
    mask = np.asarray(np.asarray(mask) != 0)
    # Masked keys get softmax weight exactly 0 (exp underflow), so attention
    # only needs the unmasked keys: pack them per batch, padded to a 128
    # multiple (even tile count); pad slots get the -1e4 bias -> exp==0.
    counts = [int(mask[b].sum()) for b in range(4)]
    nkt2 = max(2, -(-max(counts) // 128))
    nkt2 += nkt2 % 2
    SK = nkt2 * 128
    bf16 = ml_dtypes.bfloat16
    fp8 = ml_dtypes.float8_e4m3
    common = {
        "Wq8": np.ascontiguousarray(np.asarray(Wq, dtype=np.float32).T.astype(fp8)),
        "Wk8": np.ascontiguousarray(np.asarray(Wk, dtype=np.float32).T.astype(fp8)),
        "Wv8": np.ascontiguousarray(np.asarray(Wv, dtype=np.float32).T.astype(fp8)),
        "bq": np.ascontiguousarray(bq, dtype=np.float32),
        "gamma": np.ascontiguousarray(gamma, dtype=np.float32),
        "beta": np.ascontiguousarray(beta, dtype=np.float32),
    }

    # residual carries x + bv (exact: ctx/rs + bv + x == (ctx incl. bv)/rs + x)
    xres16 = (x + np.asarray(bv, dtype=np.float32)).astype(bf16)
    x8 = x.astype(fp8)
    in_maps = []
    for b in range(4):
        sel = x8[b][mask[b]]
        xkv8h = np.zeros((SK, E), dtype=fp8)
        xkv8h[: len(sel)] = sel
        xkvT8 = np.ascontiguousarray(xkv8h.T)
        mb = np.full(SK, MASK_NEG + SHIFT, dtype=np.float32)
        mb[: len(sel)] = SHIFT
        for h in range(2):
            in_maps.append(
                {
                    "xq": np.ascontiguousarray(xres16[b, h * SQ : (h + 1) * SQ]),
                    "xqT8": np.ascontiguousarray(x8[b, h * SQ : (h + 1) * SQ].T),
                    "xkvT8": xkvT8,
                    "maskbias": mb,
                    **common,
                }
            )
    apply_gb = not (
        np.all(np.asarray(gamma) == 1.0) and np.all(np.asarray(beta) == 0.0)
    )
    nc = build_nc(nkt2, apply_gb)
    nc.compile()
    res = run_bass_kernel_spmd(nc, in_maps, core_ids=list(range(8)), trace=TRACE)
    LAST_RESULTS = res
    full = np.empty((4, S, E), dtype=np.float32)
    for c in range(8):
        b, h = c // 2, c % 2
        full[b, h * SQ : (h + 1) * SQ] = res.results[c]["out"].astype(np.float32)
    return full
